# revision 9
# baseline (speedup 1.0000x reference)
"""DGCNN forward pass on 8 Trainium2 NeuronCores, data-parallel over batch.

Strategy (per core = one sample, SPMD):
  EdgeConv(x, W=[Wa|Wb], bn) = max_k leaky(bn(Wa(nbr-ctr) + Wb ctr))
  Since bn scale > 0 and leaky is monotone:
      out[n] = leaky(bn( max_k y[idx[n,k]] + z[n] ))
  with y = x @ Wa^T, z = x @ (Wb-Wa)^T.  This avoids the [N,k,2C] tensor.

  kNN selection score: S[m,j] = <x_m, x_j> - 0.5||x_j||^2 (same ranking as
  the reference's negative squared distance per row). Computed on PE in full
  fp32 via one matmul with an augmented contraction row (ones x -0.5 sq).
  Top-20 per row via 3 rounds of DVE max8/max_index8/match_replace8 (exact).

  Neighbor gather: y rows are written to DRAM; indices are rearranged into
  the 16-partition-wrapped layout via a DRAM bounce; one dma_gather per
  128-point tile fetches [128, 20, O]; DVE strided tensor_reduce(max) gives
  the neighbor max. A PE transpose accumulates it onto z^T in PSUM, and one
  ACT Prelu (bn fold: scale=g/sqrt(v+eps), bias=b-m*scale, alpha=0.2)
  produces the next layer's transposed features.

  Finale: h5 = leaky(bn5(W5 @ H)) computed per 128-output chunk in [o,n]
  layout; global max (DVE reduce) + mean (ACT accum_out) pooling; FC layers
  as [M,1] matmul chains with per-partition bn scales.
"""
import numpy as np

N = 1024
KNN = 20
NT = 8  # 128-row tiles
LAYERS = [(3, 64), (64, 64), (64, 128), (128, 256)]
EPS = 1e-5

_CACHE = {}


def _build_program():
    import concourse.bass as bass
    import concourse.bacc as bacc
    import concourse.mybir as mybir
    import concourse.tile as tile
    from concourse.bass_types import AP
    from concourse.masks import make_identity

    f32 = mybir.dt.float32
    u16 = mybir.dt.uint16
    i16 = mybir.dt.int16
    AF = mybir.ActivationFunctionType
    ALU = mybir.AluOpType
    AX = mybir.AxisListType

    nc = bacc.Bacc("TRN2", target_bir_lowering=False, debug=False)

    # ---------------- DRAM tensors ----------------
    x_d = nc.dram_tensor("x", (N, 3), f32, kind="ExternalInput")
    wa_d, wb_d, sc_d, bi_d = [], [], [], []
    for t, (C, O) in enumerate(LAYERS):
        wa_d.append(nc.dram_tensor(f"wa{t}", (C, O), f32, kind="ExternalInput"))
        wb_d.append(nc.dram_tensor(f"wb{t}", (C, O), f32, kind="ExternalInput"))
        ncol = (O + 127) // 128
        sc_d.append(nc.dram_tensor(f"sc{t}", (128, ncol), f32, kind="ExternalInput"))
        bi_d.append(nc.dram_tensor(f"bi{t}", (128, ncol), f32, kind="ExternalInput"))
    w5_d = nc.dram_tensor("w5t", (512, 1024), f32, kind="ExternalInput")
    s5_d = nc.dram_tensor("s5", (128, 8), f32, kind="ExternalInput")
    b5_d = nc.dram_tensor("b5", (128, 8), f32, kind="ExternalInput")
    fc1_d = nc.dram_tensor("fc1t", (2048, 512), f32, kind="ExternalInput")
    s6_d = nc.dram_tensor("s6", (128, 4), f32, kind="ExternalInput")
    b6_d = nc.dram_tensor("b6", (128, 4), f32, kind="ExternalInput")
    fc2_d = nc.dram_tensor("fc2t", (512, 256), f32, kind="ExternalInput")
    s7_d = nc.dram_tensor("s7", (128, 2), f32, kind="ExternalInput")
    b7_d = nc.dram_tensor("b7", (128, 2), f32, kind="ExternalInput")
    fc3_d = nc.dram_tensor("fc3t", (256, 10), f32, kind="ExternalInput")
    b3_d = nc.dram_tensor("fc3b", (10, 1), f32, kind="ExternalInput")
    out_d = nc.dram_tensor("out", (10, 1), f32, kind="ExternalOutput")

    with tile.TileContext(nc) as tc:
        with tc.tile_pool(name="const", bufs=1) as constp, \
             tc.tile_pool(name="feat", bufs=1) as featp, \
             tc.tile_pool(name="spool", bufs=NT) as spool, \
             tc.tile_pool(name="work", bufs=2) as work, \
             tc.tile_pool(name="dram", bufs=1, space="DRAM") as dpool, \
             tc.tile_pool(name="psum", bufs=2, space="PSUM") as psum:

            y_d = [dpool.tile([N, O], f32, name=f"yscr{t}", tag=f"yscr{t}")
                   for t, (C, O) in enumerate(LAYERS)]

            ident = constp.tile([128, 128], f32, name="ident")
            make_identity(nc, ident[:])
            ones_row = constp.tile([1, N], f32, name="ones_row")
            nc.vector.memset(ones_row[:], 1.0)
            ones_col = constp.tile([128, 1], f32, name="ones_col")
            nc.vector.memset(ones_col[:], 1.0)

            # layer weights in SBUF
            wa_sb, wb_sb, sc_sb, bi_sb = [], [], [], []
            for t, (C, O) in enumerate(LAYERS):
                wa = constp.tile([C, O], f32, name=f"wa_sb{t}")
                wb = constp.tile([C, O], f32, name=f"wb_sb{t}")
                ncol = (O + 127) // 128
                sc = constp.tile([128, ncol], f32, name=f"sc_sb{t}")
                bi = constp.tile([128, ncol], f32, name=f"bi_sb{t}")
                nc.sync.dma_start(out=wa[:], in_=wa_d[t][:])
                nc.sync.dma_start(out=wb[:], in_=wb_d[t][:])
                nc.sync.dma_start(out=sc[:], in_=sc_d[t][:])
                nc.sync.dma_start(out=bi[:], in_=bi_d[t][:])
                wa_sb.append(wa); wb_sb.append(wb); sc_sb.append(sc); bi_sb.append(bi)

            # persistent feature tensors (transposed layout [C, N]) + fold rows
            F1s = featp.tile([65, N], f32, name="F1s")   # L1 out + sq row
            F1o = featp.tile([65, N], f32, name="F1o")   # L1 out + ones row
            F2s = featp.tile([65, N], f32, name="F2s")
            F2o = featp.tile([65, N], f32, name="F2o")
            F3s = featp.tile([128, N], f32, name="F3s")
            sq3 = featp.tile([1, N], f32, name="sq3")
            F4a = featp.tile([128, N], f32, name="F4a")
            F4b = featp.tile([128, N], f32, name="F4b")
            X1s = featp.tile([4, N], f32, name="X1s")    # input x^T + sq row
            X1o = featp.tile([4, N], f32, name="X1o")    # input x^T + ones row

            # ---------------- phase 0: build X1 from x ----------------
            xrow = work.tile([128, 24], f32, name="xrow", bufs=1)
            for m in range(NT):
                nc.sync.dma_start(out=xrow[:, m * 3:(m + 1) * 3],
                                  in_=x_d[m * 128:(m + 1) * 128, :])
            xt_ps = psum.tile([4, N], f32, name="xt_ps", tag="z")
            for m in range(NT):
                nc.tensor.matmul(out=xt_ps[0:3, m * 128:(m + 1) * 128],
                                 lhsT=xrow[:, m * 3:(m + 1) * 3], rhs=ident[:],
                                 is_transpose=True, start=True, stop=True,
                                 skip_group_check=True)
            nc.scalar.copy(out=X1s[0:3, :], in_=xt_ps[0:3, :])
            nc.scalar.copy(out=X1o[0:3, :], in_=xt_ps[0:3, :])
            # partition base 3 is illegal for compute engines; route via DMA
            nc.sync.dma_start(out=X1o[3:4, :], in_=ones_row[:])

            def emit_sq(feat_rows, dst_row_ap):
                # dst_row_ap[0, j] = -0.5 * sum_c feat[c, j]^2
                C = feat_rows.shape[0]
                xsq = work.tile([128, N], f32, name="xsq", tag="xsq", bufs=2)
                nc.scalar.square(out=xsq[0:C, :], in_=feat_rows)
                sq_ps = psum.tile([1, N], f32, name="sq_ps", tag="z")
                for h in range(2):
                    nc.tensor.matmul(out=sq_ps[:, h * 512:(h + 1) * 512],
                                     lhsT=ones_col[0:C, :],
                                     rhs=xsq[0:C, h * 512:(h + 1) * 512],
                                     start=True, stop=True, skip_group_check=True)
                nc.scalar.activation(out=dst_row_ap, in_=sq_ps[:],
                                     func=AF.Copy, scale=-0.5)

            sqrow1 = featp.tile([1, N], f32, name="sqrow1")
            emit_sq(X1s[0:3, :], sqrow1[:])
            nc.sync.dma_start(out=X1s[3:4, :], in_=sqrow1[:])

            # layer configs: (xts/xto with inline fold) or (feat + separate sq)
            # in_feat: AP [C, N] used for y/z matmuls
            layer_in = [
                dict(xts=X1s, xto=X1o, Cf=4, feat=X1s[0:3, :]),
                dict(xts=F1s, xto=F1o, Cf=65, feat=F1s[0:64, :]),
                dict(xts=F2s, xto=F2o, Cf=65, feat=F2s[0:64, :]),
                dict(sep=(F3s, sq3), feat=F3s[:]),
            ]
            layer_out = [
                dict(feats=[F1s[0:64, :]], sqrow=F1s[64:65, :],
                     ocopy=(F1o, 64)),
                dict(feats=[F2s[0:64, :]], sqrow=F2s[64:65, :],
                     ocopy=(F2o, 64)),
                dict(feats=[F3s[:]], sqrow=sq3[:], ocopy=None),
                dict(feats=[F4a[:], F4b[:]], sqrow=None, ocopy=None),
            ]

            for t, (C, O) in enumerate(LAYERS):
                li, lo = layer_in[t], layer_out[t]
                feat_in = li["feat"]
                nchunk = (O + 127) // 128

                # ---- S = G - 0.5 sq[j] : PE + ACT evict ----
                s_tiles = []
                for m in range(NT):
                    g_ps = psum.tile([128, N], f32, name="g_ps", tag="g")
                    for h in range(2):
                        if "xts" in li:
                            Cf = li["Cf"]
                            nc.tensor.matmul(
                                out=g_ps[:, h * 512:(h + 1) * 512],
                                lhsT=li["xto"][0:Cf, m * 128:(m + 1) * 128],
                                rhs=li["xts"][0:Cf, h * 512:(h + 1) * 512],
                                start=True, stop=True, skip_group_check=True)
                        else:
                            feat, sqr = li["sep"]
                            nc.tensor.matmul(
                                out=g_ps[:, h * 512:(h + 1) * 512],
                                lhsT=feat[:, m * 128:(m + 1) * 128],
                                rhs=feat[:, h * 512:(h + 1) * 512],
                                start=True, stop=False, skip_group_check=True)
                            nc.tensor.matmul(
                                out=g_ps[:, h * 512:(h + 1) * 512],
                                lhsT=ones_row[:, m * 128:(m + 1) * 128],
                                rhs=sqr[:, h * 512:(h + 1) * 512],
                                start=False, stop=True, skip_group_check=True)
                    s_sb = spool.tile([128, N], f32, name="s_sb", tag="S")
                    nc.scalar.copy(out=s_sb[:], in_=g_ps[:])
                    s_tiles.append(s_sb)

                # ---- y = x @ Wa^T -> rows -> DRAM ----
                for m in range(NT):
                    y_ps = psum.tile([128, O], f32, name="y_ps", tag="z")
                    nc.tensor.matmul(out=y_ps[:],
                                     lhsT=feat_in[:, m * 128:(m + 1) * 128],
                                     rhs=wa_sb[t][:], start=True, stop=True,
                                     skip_group_check=True)
                    y_sb = work.tile([128, 256], f32, name="y_sb", tag="ysb")
                    nc.scalar.copy(out=y_sb[:, 0:O], in_=y_ps[:])
                    nc.sync.dma_start(out=y_d[t][m * 128:(m + 1) * 128, :],
                                      in_=y_sb[:, 0:O])

                # ---- z^T chunks in PSUM ----
                z_ps = []
                for p in range(nchunk):
                    op = min(128, O - p * 128)
                    zp = psum.tile([128, N], f32, name="z_ps", tag="g", bufs=2)
                    for h in range(2):
                        nc.tensor.matmul(out=zp[0:op, h * 512:(h + 1) * 512],
                                         lhsT=wb_sb[t][:, p * 128:p * 128 + op],
                                         rhs=feat_in[:, h * 512:(h + 1) * 512],
                                         start=True, stop=False,
                                         skip_group_check=True)
                    z_ps.append((zp, op))

                # ---- topk + idx wrap + gather + nbr max + transpose-accum ----
                for m in range(NT):
                    s_sb = s_tiles[m]
                    v24 = work.tile([128, 24], f32, name="v24", tag="v24", bufs=4)
                    i24 = work.tile([128, 24], u16, name="i24", tag="i24", bufs=4)
                    for r in range(3):
                        nc.vector.max(out=v24[:, r * 8:(r + 1) * 8], in_=s_sb[:])
                        nc.vector.max_index(out=i24[:, r * 8:(r + 1) * 8],
                                            in_max=v24[:, r * 8:(r + 1) * 8],
                                            in_values=s_sb[:])
                        if r < 2:
                            nc.vector.match_replace(
                                out=s_sb[:], in_to_replace=v24[:, r * 8:(r + 1) * 8],
                                in_values=s_sb[:], imm_value=-1e30)

                    # wrapped[r, 8k+b] = i24[16b+r, k]
                    wd = dpool.tile([16, KNN * 8], u16, name="wd", tag="wrap",
                                    bufs=3)
                    for b in range(8):
                        nc.sync.dma_start(
                            out=AP(wd.tensor, b, [[KNN * 8, 16], [8, KNN]]),
                            in_=i24[16 * b:16 * (b + 1), 0:KNN])
                    idx_w = work.tile([128, KNN * 8], u16, name="idx_w",
                                      tag="idxw", bufs=2)
                    for c in range(8):
                        nc.sync.dma_start(out=idx_w[16 * c:16 * (c + 1), :],
                                          in_=wd[:])

                    gbuf = work.tile([128, KNN, O], f32, name="gbuf",
                                     tag="gbuf", bufs=2)
                    nc.gpsimd.dma_gather(
                        out_ap=gbuf[:], in_ap=y_d[t][:],
                        idxs_ap=idx_w[:].bitcast(i16),
                        num_idxs=KNN * 128, num_idxs_reg=KNN * 128,
                        elem_size=O, single_packet=False)

                    m_sb = work.tile([128, 256], f32, name="m_sb", tag="msb",
                                     bufs=3)
                    nc.vector.tensor_reduce(
                        out=m_sb[:, 0:O], in_=gbuf[:].rearrange("p k o -> p o k"),
                        axis=AX.X, op=ALU.max)

                    for p in range(nchunk):
                        zp, op = z_ps[p]
                        nc.tensor.matmul(out=zp[0:op, m * 128:(m + 1) * 128],
                                         lhsT=m_sb[:, p * 128:p * 128 + op],
                                         rhs=ident[:], is_transpose=True,
                                         start=False, stop=(m == NT - 1),
                                         skip_group_check=True)

                # ---- bn + leaky -> next features ----
                for p in range(nchunk):
                    zp, op = z_ps[p]
                    dst = lo["feats"][p]
                    nc.scalar.activation(out=dst, in_=zp[0:op, :], func=AF.Prelu,
                                         bias=bi_sb[t][0:op, p:p + 1],
                                         scale=sc_sb[t][0:op, p:p + 1], alpha=0.2)

                if lo["ocopy"] is not None:
                    oten, crows = lo["ocopy"]
                    nc.scalar.copy(out=oten[0:crows, :], in_=lo["feats"][0])
                    nc.vector.memset(oten[crows:crows + 1, :], 1.0)
                if lo["sqrow"] is not None:
                    if len(lo["feats"]) == 1:
                        emit_sq(lo["feats"][0], lo["sqrow"])
                    else:
                        raise AssertionError("multi-chunk sq not needed")

            # ---------------- finale ----------------
            # h5^T chunks: [o5 (128), n (1024)] = W5[o5, :512] @ H
            w5_0a = constp.tile([64, 1024], f32, name="w5_0a")
            w5_0b = constp.tile([64, 1024], f32, name="w5_0b")
            nc.sync.dma_start(out=w5_0a[:], in_=w5_d[0:64, :])
            nc.sync.dma_start(out=w5_0b[:], in_=w5_d[64:128, :])
            w5_sb = [None] + [constp.tile([128, 1024], f32, name=f"w5_sb{kc}")
                              for kc in range(1, 4)]
            for kc in range(1, 4):
                nc.sync.dma_start(out=w5_sb[kc][:],
                                  in_=w5_d[kc * 128:(kc + 1) * 128, :])
            s5_sb = constp.tile([128, 8], f32, name="s5_sb")
            b5_sb = constp.tile([128, 8], f32, name="b5_sb")
            nc.sync.dma_start(out=s5_sb[:], in_=s5_d[:])
            nc.sync.dma_start(out=b5_sb[:], in_=b5_d[:])

            hk = [(F1s[0:64, :], w5_0a[:]),
                  (F2s[0:64, :], w5_0b[:]),
                  (F3s[:], w5_sb[1][:]),
                  (F4a[:], w5_sb[2][:]),
                  (F4b[:], w5_sb[3][:])]
            gcat = constp.tile([128, 16], f32, name="gcat")
            for m in range(NT):
                h_ps = psum.tile([128, N], f32, name="h_ps", tag="g")
                for h in range(2):
                    for j, (fk, wk) in enumerate(hk):
                        cc = fk.shape[0]
                        nc.tensor.matmul(out=h_ps[:, h * 512:(h + 1) * 512],
                                         lhsT=wk[:, m * 128:(m + 1) * 128],
                                         rhs=fk[:, h * 512:(h + 1) * 512],
                                         start=(j == 0), stop=(j == len(hk) - 1),
                                         skip_group_check=True)
                h_sb = work.tile([128, N], f32, name="h_sb", tag="hsb", bufs=2)
                nc.scalar.activation(out=h_sb[:], in_=h_ps[:], func=AF.Prelu,
                                     bias=b5_sb[:, m:m + 1], scale=s5_sb[:, m:m + 1],
                                     alpha=0.2, accum_out=gcat[:, 8 + m:9 + m])
                nc.vector.tensor_reduce(out=gcat[:, m:m + 1], in_=h_sb[:],
                                        axis=AX.X, op=ALU.max)

            # fc1: [512] out in 4 chunks of [128, 1]
            fc1_sb = [constp.tile([128, 512], f32, name=f"fc1_sb{kc}")
                      for kc in range(16)]
            for kc in range(16):
                nc.sync.dma_start(out=fc1_sb[kc][:],
                                  in_=fc1_d[kc * 128:(kc + 1) * 128, :])
            s6_sb = constp.tile([128, 4], f32, name="s6_sb")
            b6_sb = constp.tile([128, 4], f32, name="b6_sb")
            nc.sync.dma_start(out=s6_sb[:], in_=s6_d[:])
            nc.sync.dma_start(out=b6_sb[:], in_=b6_d[:])
            g1 = constp.tile([128, 4], f32, name="g1")
            for m4 in range(4):
                f_ps = psum.tile([128, 1], f32, name="f_ps", tag="z")
                for kc in range(16):
                    nc.tensor.matmul(out=f_ps[:],
                                     lhsT=fc1_sb[kc][:, m4 * 128:(m4 + 1) * 128],
                                     rhs=gcat[:, kc:kc + 1],
                                     start=(kc == 0), stop=(kc == 15),
                                     skip_group_check=True)
                nc.scalar.activation(out=g1[:, m4:m4 + 1], in_=f_ps[:],
                                     func=AF.Prelu, bias=b6_sb[:, m4:m4 + 1],
                                     scale=s6_sb[:, m4:m4 + 1], alpha=0.2)

            fc2_sb = [constp.tile([128, 256], f32, name=f"fc2_sb{kc}")
                      for kc in range(4)]
            for kc in range(4):
                nc.sync.dma_start(out=fc2_sb[kc][:],
                                  in_=fc2_d[kc * 128:(kc + 1) * 128, :])
            s7_sb = constp.tile([128, 2], f32, name="s7_sb")
            b7_sb = constp.tile([128, 2], f32, name="b7_sb")
            nc.sync.dma_start(out=s7_sb[:], in_=s7_d[:])
            nc.sync.dma_start(out=b7_sb[:], in_=b7_d[:])
            g2 = constp.tile([128, 2], f32, name="g2")
            for m2 in range(2):
                f_ps = psum.tile([128, 1], f32, name="f_ps2", tag="z")
                for kc in range(4):
                    nc.tensor.matmul(out=f_ps[:],
                                     lhsT=fc2_sb[kc][:, m2 * 128:(m2 + 1) * 128],
                                     rhs=g1[:, kc:kc + 1],
                                     start=(kc == 0), stop=(kc == 3),
                                     skip_group_check=True)
                nc.scalar.activation(out=g2[:, m2:m2 + 1], in_=f_ps[:],
                                     func=AF.Prelu, bias=b7_sb[:, m2:m2 + 1],
                                     scale=s7_sb[:, m2:m2 + 1], alpha=0.2)

            fc3_sb = [constp.tile([128, 10], f32, name=f"fc3_sb{kc}")
                      for kc in range(2)]
            for kc in range(2):
                nc.sync.dma_start(out=fc3_sb[kc][:],
                                  in_=fc3_d[kc * 128:(kc + 1) * 128, :])
            b3_sb = constp.tile([10, 1], f32, name="b3_sb")
            nc.sync.dma_start(out=b3_sb[:], in_=b3_d[:])
            l_ps = psum.tile([10, 1], f32, name="l_ps", tag="z")
            for kc in range(2):
                nc.tensor.matmul(out=l_ps[:], lhsT=fc3_sb[kc][:],
                                 rhs=g2[:, kc:kc + 1],
                                 start=(kc == 0), stop=(kc == 1),
                                 skip_group_check=True)
            out_sb = constp.tile([10, 1], f32, name="out_sb")
            nc.scalar.activation(out=out_sb[:], in_=l_ps[:],
                                 func=AF.Identity, bias=b3_sb[:], scale=1.0)
            nc.sync.dma_start(out=out_d[:], in_=out_sb[:])

    nc.compile()
    return nc


def _prep_shared(params):
    """Host-side weight prep -> dict of numpy arrays for the bass program."""
    f = np.float32
    shared = {}

    def bn_fold(bn):
        g, b, m, v = [np.asarray(a, np.float64) for a in bn]
        s = g / np.sqrt(v + EPS)
        beta = b - m * s
        return s.astype(f), beta.astype(f)

    def col128(v):
        O = v.shape[0]
        ncol = (O + 127) // 128
        out = np.zeros((128, ncol), f)
        for p in range(ncol):
            op = min(128, O - p * 128)
            out[0:op, p] = v[p * 128:p * 128 + op]
        return out

    wkeys = [("w1", "bn1"), ("w2", "bn2"), ("w3", "bn3"), ("w4", "bn4")]
    for t, (wk, bk) in enumerate(wkeys):
        w = np.asarray(params[wk], f)          # [O, 2C]
        C = w.shape[1] // 2
        wa = w[:, :C]
        wbma = w[:, C:] - wa
        shared[f"wa{t}"] = np.ascontiguousarray(wa.T)
        shared[f"wb{t}"] = np.ascontiguousarray(wbma.T)
        s, beta = bn_fold(params[bk])
        shared[f"sc{t}"] = col128(s)
        shared[f"bi{t}"] = col128(beta)

    shared["w5t"] = np.ascontiguousarray(np.asarray(params["w5"], f).T)  # [512,1024]
    s5, b5 = bn_fold(params["bn5"])
    shared["s5"] = s5.reshape(8, 128).T.copy()
    shared["b5"] = b5.reshape(8, 128).T.copy()

    fc1 = np.asarray(params["fc1"], f).copy()  # [512, 2048]
    fc1[:, 1024:] *= 1.0 / N                   # fold mean-pool divisor
    shared["fc1t"] = np.ascontiguousarray(fc1.T)  # [2048, 512]
    s6, b6 = bn_fold(params["bn6"])
    shared["s6"] = s6.reshape(4, 128).T.copy()
    shared["b6"] = b6.reshape(4, 128).T.copy()

    shared["fc2t"] = np.ascontiguousarray(np.asarray(params["fc2"], f).T)
    s7, b7 = bn_fold(params["bn7"])
    shared["s7"] = s7.reshape(2, 128).T.copy()
    shared["b7"] = b7.reshape(2, 128).T.copy()

    shared["fc3t"] = np.ascontiguousarray(np.asarray(params["fc3_w"], f).T)
    shared["fc3b"] = np.asarray(params["fc3_b"], f).reshape(10, 1).copy()
    return shared


def kernel(x, params):
    from concourse import bass_utils

    x = np.asarray(x, np.float32)
    B = x.shape[0]
    assert x.shape == (8, N, 3)

    if "nc" not in _CACHE:
        _CACHE["nc"] = _build_program()
    nc = _CACHE["nc"]

    shared = _prep_shared(params)
    in_maps = [dict(shared, x=np.ascontiguousarray(x[i])) for i in range(B)]
    res = bass_utils.run_bass_kernel_spmd(nc, in_maps, core_ids=list(range(B)))
    return np.stack([r["out"].reshape(10) for r in res.results])


# revision 16
# speedup vs baseline: 1.1393x; 1.1393x over previous
"""DGCNN forward pass on 8 Trainium2 NeuronCores, data-parallel over batch.

Strategy (per core = one sample, SPMD):
  EdgeConv(x, W=[Wa|Wb], bn) = max_k leaky(bn(Wa(nbr-ctr) + Wb ctr))
  Since bn scale > 0 and leaky is monotone:
      out[n] = leaky(bn( max_k y[idx[n,k]] + z[n] ))
  with y = x @ Wa^T, z = x @ (Wb-Wa)^T.  This avoids the [N,k,2C] tensor.

  kNN selection score: S[m,j] = <x_m, x_j> - 0.5||x_j||^2 (same ranking as
  the reference's negative squared distance per row). Computed on PE in full
  fp32 via one matmul with an augmented contraction row (ones x -0.5 sq).
  Top-20 per row via 3 rounds of DVE max8/max_index8/match_replace8 (exact).

  Neighbor gather: y rows are written to DRAM; indices are rearranged into
  the 16-partition-wrapped layout via a DRAM bounce; one dma_gather per
  128-point tile fetches [128, 20, O]; DVE strided tensor_reduce(max) gives
  the neighbor max. A PE transpose accumulates it onto z^T in PSUM, and one
  ACT Prelu (bn fold: scale=g/sqrt(v+eps), bias=b-m*scale, alpha=0.2)
  produces the next layer's transposed features.

  Finale: h5 = leaky(bn5(W5 @ H)) computed per 128-output chunk in [o,n]
  layout; global max (DVE reduce) + mean (ACT accum_out) pooling; FC layers
  as [M,1] matmul chains with per-partition bn scales.
"""
import numpy as np

N = 1024
KNN = 20
NT = 8  # 128-row tiles
LAYERS = [(3, 64), (64, 64), (64, 128), (128, 256)]
EPS = 1e-5

_CACHE = {}


def _build_program():
    import concourse.bass as bass
    import concourse.bacc as bacc
    import concourse.mybir as mybir
    import concourse.tile as tile
    from concourse.bass_types import AP
    from concourse.masks import make_identity

    f32 = mybir.dt.float32
    u16 = mybir.dt.uint16
    i16 = mybir.dt.int16
    AF = mybir.ActivationFunctionType
    ALU = mybir.AluOpType
    AX = mybir.AxisListType

    nc = bacc.Bacc("TRN2", target_bir_lowering=False, debug=False)

    # ---------------- DRAM tensors ----------------
    x_d = nc.dram_tensor("x", (N, 3), f32, kind="ExternalInput")
    wa_d, wb_d, sc_d, bi_d = [], [], [], []
    for t, (C, O) in enumerate(LAYERS):
        wa_d.append(nc.dram_tensor(f"wa{t}", (C, O), f32, kind="ExternalInput"))
        wb_d.append(nc.dram_tensor(f"wb{t}", (C, O), f32, kind="ExternalInput"))
        ncol = (O + 127) // 128
        sc_d.append(nc.dram_tensor(f"sc{t}", (128, ncol), f32, kind="ExternalInput"))
        bi_d.append(nc.dram_tensor(f"bi{t}", (128, ncol), f32, kind="ExternalInput"))
    w5_d = nc.dram_tensor("w5t", (512, 1024), f32, kind="ExternalInput")
    s5_d = nc.dram_tensor("s5", (128, 8), f32, kind="ExternalInput")
    b5_d = nc.dram_tensor("b5", (128, 8), f32, kind="ExternalInput")
    fc1_d = nc.dram_tensor("fc1t", (2048, 512), f32, kind="ExternalInput")
    s6_d = nc.dram_tensor("s6", (128, 4), f32, kind="ExternalInput")
    b6_d = nc.dram_tensor("b6", (128, 4), f32, kind="ExternalInput")
    fc2_d = nc.dram_tensor("fc2t", (512, 256), f32, kind="ExternalInput")
    s7_d = nc.dram_tensor("s7", (128, 2), f32, kind="ExternalInput")
    b7_d = nc.dram_tensor("b7", (128, 2), f32, kind="ExternalInput")
    fc3_d = nc.dram_tensor("fc3t", (256, 10), f32, kind="ExternalInput")
    b3_d = nc.dram_tensor("fc3b", (10, 1), f32, kind="ExternalInput")
    out_d = nc.dram_tensor("out", (10, 1), f32, kind="ExternalOutput")

    with tile.TileContext(nc) as tc:
        with tc.tile_pool(name="const", bufs=1) as constp, \
             tc.tile_pool(name="feat", bufs=1) as featp, \
             tc.tile_pool(name="spool", bufs=NT) as spool, \
             tc.tile_pool(name="work", bufs=2) as work, \
             tc.tile_pool(name="dram", bufs=1, space="DRAM") as dpool, \
             tc.tile_pool(name="psum", bufs=2, space="PSUM") as psum:

            y_d = [dpool.tile([N, O], f32, name=f"yscr{t}", tag=f"yscr{t}")
                   for t, (C, O) in enumerate(LAYERS)]

            ident = constp.tile([128, 128], f32, name="ident")
            make_identity(nc, ident[:])
            ones_row = constp.tile([1, N], f32, name="ones_row")
            nc.vector.memset(ones_row[:], 1.0)
            ones_col = constp.tile([128, 1], f32, name="ones_col")
            nc.vector.memset(ones_col[:], 1.0)

            # layer weights in SBUF
            wa_sb, wb_sb, sc_sb, bi_sb = [], [], [], []
            for t, (C, O) in enumerate(LAYERS):
                wa = constp.tile([C, O], f32, name=f"wa_sb{t}")
                wb = constp.tile([C, O], f32, name=f"wb_sb{t}")
                ncol = (O + 127) // 128
                sc = constp.tile([128, ncol], f32, name=f"sc_sb{t}")
                bi = constp.tile([128, ncol], f32, name=f"bi_sb{t}")
                nc.sync.dma_start(out=wa[:], in_=wa_d[t][:])
                nc.sync.dma_start(out=wb[:], in_=wb_d[t][:])
                nc.sync.dma_start(out=sc[:], in_=sc_d[t][:])
                nc.sync.dma_start(out=bi[:], in_=bi_d[t][:])
                wa_sb.append(wa); wb_sb.append(wb); sc_sb.append(sc); bi_sb.append(bi)

            # persistent feature tensors (transposed layout [C, N]) + fold rows
            F1s = featp.tile([65, N], f32, name="F1s")   # L1 out + sq row
            F1o = featp.tile([65, N], f32, name="F1o")   # L1 out + ones row
            F2s = featp.tile([65, N], f32, name="F2s")
            F2o = featp.tile([65, N], f32, name="F2o")
            F3s = featp.tile([128, N], f32, name="F3s")
            sq3 = featp.tile([1, N], f32, name="sq3")
            F4a = featp.tile([128, N], f32, name="F4a")
            F4b = featp.tile([128, N], f32, name="F4b")
            X1s = featp.tile([4, N], f32, name="X1s")    # input x^T + sq row
            X1o = featp.tile([4, N], f32, name="X1o")    # input x^T + ones row

            # ---------------- phase 0: build X1 from x ----------------
            xrow = work.tile([128, 24], f32, name="xrow", bufs=1)
            for m in range(NT):
                nc.sync.dma_start(out=xrow[:, m * 3:(m + 1) * 3],
                                  in_=x_d[m * 128:(m + 1) * 128, :])
            xt_ps = psum.tile([4, N], f32, name="xt_ps", tag="z")
            for m in range(NT):
                nc.tensor.matmul(out=xt_ps[0:3, m * 128:(m + 1) * 128],
                                 lhsT=xrow[:, m * 3:(m + 1) * 3], rhs=ident[:],
                                 is_transpose=True, start=True, stop=True,
                                 skip_group_check=True)
            nc.scalar.copy(out=X1s[0:3, :], in_=xt_ps[0:3, :])
            nc.scalar.copy(out=X1o[0:3, :], in_=xt_ps[0:3, :])
            # partition base 3 is illegal for compute engines; route via DMA
            nc.sync.dma_start(out=X1o[3:4, :], in_=ones_row[:])

            def emit_sq(feat_rows, dst_row_ap):
                # dst_row_ap[0, j] = -0.5 * sum_c feat[c, j]^2
                C = feat_rows.shape[0]
                xsq = work.tile([128, N], f32, name="xsq", tag="xsq", bufs=2)
                nc.scalar.square(out=xsq[0:C, :], in_=feat_rows)
                sq_ps = psum.tile([1, N], f32, name="sq_ps", tag="z")
                for h in range(2):
                    nc.tensor.matmul(out=sq_ps[:, h * 512:(h + 1) * 512],
                                     lhsT=ones_col[0:C, :],
                                     rhs=xsq[0:C, h * 512:(h + 1) * 512],
                                     start=True, stop=True, skip_group_check=True)
                nc.scalar.activation(out=dst_row_ap, in_=sq_ps[:],
                                     func=AF.Copy, scale=-0.5)

            sqrow1 = featp.tile([1, N], f32, name="sqrow1")
            emit_sq(X1s[0:3, :], sqrow1[:])
            nc.sync.dma_start(out=X1s[3:4, :], in_=sqrow1[:])

            # layer configs: (xts/xto with inline fold) or (feat + separate sq)
            # in_feat: AP [C, N] used for y/z matmuls
            layer_in = [
                dict(xts=X1s, xto=X1o, Cf=4, feat=X1s[0:3, :]),
                dict(xts=F1s, xto=F1o, Cf=65, feat=F1s[0:64, :]),
                dict(xts=F2s, xto=F2o, Cf=65, feat=F2s[0:64, :]),
                dict(sep=(F3s, sq3), feat=F3s[:]),
            ]
            layer_out = [
                dict(feats=[F1s[0:64, :]], sqrow=F1s[64:65, :],
                     ocopy=(F1o, 64)),
                dict(feats=[F2s[0:64, :]], sqrow=F2s[64:65, :],
                     ocopy=(F2o, 64)),
                dict(feats=[F3s[:]], sqrow=sq3[:], ocopy=None),
                dict(feats=[F4a[:], F4b[:]], sqrow=None, ocopy=None),
            ]

            for t, (C, O) in enumerate(LAYERS):
                li, lo = layer_in[t], layer_out[t]
                feat_in = li["feat"]
                nchunk = (O + 127) // 128

                # ---- S = G - 0.5 sq[j] : PE + ACT evict ----
                s_tiles = []
                for m in range(NT):
                    g_ps = psum.tile([128, N], f32, name="g_ps", tag="g")
                    for h in range(2):
                        if "xts" in li:
                            Cf = li["Cf"]
                            nc.tensor.matmul(
                                out=g_ps[:, h * 512:(h + 1) * 512],
                                lhsT=li["xto"][0:Cf, m * 128:(m + 1) * 128],
                                rhs=li["xts"][0:Cf, h * 512:(h + 1) * 512],
                                start=True, stop=True, skip_group_check=True)
                        else:
                            feat, sqr = li["sep"]
                            nc.tensor.matmul(
                                out=g_ps[:, h * 512:(h + 1) * 512],
                                lhsT=feat[:, m * 128:(m + 1) * 128],
                                rhs=feat[:, h * 512:(h + 1) * 512],
                                start=True, stop=False, skip_group_check=True)
                            nc.tensor.matmul(
                                out=g_ps[:, h * 512:(h + 1) * 512],
                                lhsT=ones_row[:, m * 128:(m + 1) * 128],
                                rhs=sqr[:, h * 512:(h + 1) * 512],
                                start=False, stop=True, skip_group_check=True)
                    s_sb = spool.tile([128, N], f32, name="s_sb", tag="S")
                    nc.scalar.copy(out=s_sb[:], in_=g_ps[:])
                    v24 = work.tile([128, 24], f32, name="v24", tag="v24", bufs=8)
                    i24 = work.tile([128, 24], u16, name="i24", tag="i24", bufs=8)
                    for r in range(3):
                        nc.vector.max(out=v24[:, r * 8:(r + 1) * 8], in_=s_sb[:])
                        nc.vector.max_index(out=i24[:, r * 8:(r + 1) * 8],
                                            in_max=v24[:, r * 8:(r + 1) * 8],
                                            in_values=s_sb[:])
                        if r < 2:
                            nc.vector.match_replace(
                                out=s_sb[:], in_to_replace=v24[:, r * 8:(r + 1) * 8],
                                in_values=s_sb[:], imm_value=-1e30)
                    s_tiles.append((s_sb, i24))

                # ---- y = x @ Wa^T -> rows -> DRAM ----
                for m in range(NT):
                    y_ps = psum.tile([128, O], f32, name="y_ps", tag="z")
                    nc.tensor.matmul(out=y_ps[:],
                                     lhsT=feat_in[:, m * 128:(m + 1) * 128],
                                     rhs=wa_sb[t][:], start=True, stop=True,
                                     skip_group_check=True)
                    y_sb = work.tile([128, 256], f32, name="y_sb", tag="ysb")
                    nc.scalar.copy(out=y_sb[:, 0:O], in_=y_ps[:])
                    nc.sync.dma_start(out=y_d[t][m * 128:(m + 1) * 128, :],
                                      in_=y_sb[:, 0:O])

                # ---- z^T chunks in PSUM ----
                z_ps = []
                for p in range(nchunk):
                    op = min(128, O - p * 128)
                    zp = psum.tile([128, N], f32, name="z_ps", tag="g", bufs=2)
                    for h in range(2):
                        nc.tensor.matmul(out=zp[0:op, h * 512:(h + 1) * 512],
                                         lhsT=wb_sb[t][:, p * 128:p * 128 + op],
                                         rhs=feat_in[:, h * 512:(h + 1) * 512],
                                         start=True, stop=False,
                                         skip_group_check=True)
                    z_ps.append((zp, op))

                # ---- topk + idx wrap + gather + nbr max + transpose-accum ----
                for m in range(NT):
                    _s_sb, i24 = s_tiles[m]
                    # wrapped[r, 8k+b] = i24[16b+r, k]
                    wd = dpool.tile([16, KNN * 8], u16, name="wd", tag="wrap",
                                    bufs=3)
                    for b in range(8):
                        eng = nc.sync if b % 2 == 0 else nc.scalar
                        eng.dma_start(
                            out=AP(wd.tensor, b, [[KNN * 8, 16], [8, KNN]]),
                            in_=i24[16 * b:16 * (b + 1), 0:KNN])
                    idx_w = work.tile([128, KNN * 8], u16, name="idx_w",
                                      tag="idxw", bufs=2)
                    for c in range(8):
                        eng = nc.sync if c % 2 == 0 else nc.scalar
                        eng.dma_start(out=idx_w[16 * c:16 * (c + 1), :],
                                      in_=wd[:])

                    gbuf = work.tile([128, KNN, O], f32, name="gbuf",
                                     tag="gbuf", bufs=2)
                    nc.gpsimd.dma_gather(
                        out_ap=gbuf[:], in_ap=y_d[t][:],
                        idxs_ap=idx_w[:].bitcast(i16),
                        num_idxs=KNN * 128, num_idxs_reg=KNN * 128,
                        elem_size=O, single_packet=False)

                    m_sb = work.tile([128, 256], f32, name="m_sb", tag="msb",
                                     bufs=3)
                    nc.vector.tensor_reduce(
                        out=m_sb[:, 0:O], in_=gbuf[:].rearrange("p k o -> p o k"),
                        axis=AX.X, op=ALU.max)

                    for p in range(nchunk):
                        zp, op = z_ps[p]
                        nc.tensor.matmul(out=zp[0:op, m * 128:(m + 1) * 128],
                                         lhsT=m_sb[:, p * 128:p * 128 + op],
                                         rhs=ident[:], is_transpose=True,
                                         start=False, stop=(m == NT - 1),
                                         skip_group_check=True)

                # ---- bn + leaky -> next features ----
                for p in range(nchunk):
                    zp, op = z_ps[p]
                    dst = lo["feats"][p]
                    nc.scalar.activation(out=dst, in_=zp[0:op, :], func=AF.Prelu,
                                         bias=bi_sb[t][0:op, p:p + 1],
                                         scale=sc_sb[t][0:op, p:p + 1], alpha=0.2)

                if lo["ocopy"] is not None:
                    oten, crows = lo["ocopy"]
                    nc.scalar.copy(out=oten[0:crows, :], in_=lo["feats"][0])
                    nc.vector.memset(oten[crows:crows + 1, :], 1.0)
                if lo["sqrow"] is not None:
                    if len(lo["feats"]) == 1:
                        emit_sq(lo["feats"][0], lo["sqrow"])
                    else:
                        raise AssertionError("multi-chunk sq not needed")

            # ---------------- finale ----------------
            # h5^T chunks in f32r (post-selection precision is plenty):
            # round features + weights to f32r via ACT copies.
            f32r = mybir.dt.float32r
            w5r = []
            for kc, (lo_r, hi_r) in enumerate([(0, 64), (64, 128), (128, 256),
                                               (256, 384), (384, 512)]):
                rows = hi_r - lo_r
                stage = work.tile([128, 1024], f32, name="w5stage", tag="hsb",
                                  bufs=2)
                nc.sync.dma_start(out=stage[0:rows, :], in_=w5_d[lo_r:hi_r, :])
                wr = spool.tile([128, 1024], f32r, name=f"w5r{kc}", tag="S")
                nc.scalar.copy(out=wr[0:rows, :], in_=stage[0:rows, :])
                w5r.append(wr[0:rows, :])
            fr = []
            for src, rows, nm, tg in [(F1s, 64, "F1r", "S"),
                                      (F2s, 64, "F2r", "S"),
                                      (F3s, 128, "F3r", "S"),
                                      (F4a, 128, "F4r", "gbuf"),
                                      (F4b, 128, "F4s", "gbuf")]:
                pool_ = spool if tg == "S" else work
                t_r = pool_.tile([128, 1024], f32r, name=nm, tag=tg)
                nc.scalar.copy(out=t_r[0:rows, :], in_=src[0:rows, :])
                fr.append(t_r[0:rows, :])
            s5_sb = constp.tile([128, 8], f32, name="s5_sb")
            b5_sb = constp.tile([128, 8], f32, name="b5_sb")
            nc.sync.dma_start(out=s5_sb[:], in_=s5_d[:])
            nc.sync.dma_start(out=b5_sb[:], in_=b5_d[:])

            hk = list(zip(fr, w5r))
            gcat = constp.tile([128, 16], f32, name="gcat")
            for m in range(NT):
                h_ps = psum.tile([128, N], f32, name="h_ps", tag="g")
                for h in range(2):
                    for j, (fk, wk) in enumerate(hk):
                        cc = fk.shape[0]
                        nc.tensor.matmul(out=h_ps[:, h * 512:(h + 1) * 512],
                                         lhsT=wk[:, m * 128:(m + 1) * 128],
                                         rhs=fk[:, h * 512:(h + 1) * 512],
                                         start=(j == 0), stop=(j == len(hk) - 1),
                                         skip_group_check=True)
                h_sb = work.tile([128, N], f32, name="h_sb", tag="hsb", bufs=2)
                nc.scalar.activation(out=h_sb[:], in_=h_ps[:], func=AF.Prelu,
                                     bias=b5_sb[:, m:m + 1], scale=s5_sb[:, m:m + 1],
                                     alpha=0.2, accum_out=gcat[:, 8 + m:9 + m])
                nc.vector.tensor_reduce(out=gcat[:, m:m + 1], in_=h_sb[:],
                                        axis=AX.X, op=ALU.max)

            # fc1: [512] out in 4 chunks of [128, 1]; stream the 16 K-tiles
            s6_sb = constp.tile([128, 4], f32, name="s6_sb")
            b6_sb = constp.tile([128, 4], f32, name="b6_sb")
            nc.sync.dma_start(out=s6_sb[:], in_=s6_d[:])
            nc.sync.dma_start(out=b6_sb[:], in_=b6_d[:])
            g1 = constp.tile([128, 4], f32, name="g1")
            for half in range(2):
                fc_ps = [psum.tile([128, 1], f32, name=f"fc_ps{half}{i}",
                                   tag="z") for i in range(2)]
                for kc in range(16):
                    fct = work.tile([128, 256], f32, name="fct", tag="ysb",
                                    bufs=2)
                    nc.sync.dma_start(
                        out=fct[:],
                        in_=fc1_d[kc * 128:(kc + 1) * 128,
                                  half * 256:(half + 1) * 256])
                    for i in range(2):
                        m4 = half * 2 + i
                        nc.tensor.matmul(out=fc_ps[i][:],
                                         lhsT=fct[:, i * 128:(i + 1) * 128],
                                         rhs=gcat[:, kc:kc + 1],
                                         start=(kc == 0), stop=(kc == 15),
                                         skip_group_check=True)
                for i in range(2):
                    m4 = half * 2 + i
                    nc.scalar.activation(out=g1[:, m4:m4 + 1], in_=fc_ps[i][:],
                                         func=AF.Prelu, bias=b6_sb[:, m4:m4 + 1],
                                         scale=s6_sb[:, m4:m4 + 1], alpha=0.2)

            fc2_sb = [constp.tile([128, 256], f32, name=f"fc2_sb{kc}")
                      for kc in range(4)]
            for kc in range(4):
                nc.sync.dma_start(out=fc2_sb[kc][:],
                                  in_=fc2_d[kc * 128:(kc + 1) * 128, :])
            s7_sb = constp.tile([128, 2], f32, name="s7_sb")
            b7_sb = constp.tile([128, 2], f32, name="b7_sb")
            nc.sync.dma_start(out=s7_sb[:], in_=s7_d[:])
            nc.sync.dma_start(out=b7_sb[:], in_=b7_d[:])
            g2 = constp.tile([128, 2], f32, name="g2")
            for m2 in range(2):
                f_ps = psum.tile([128, 1], f32, name="f_ps2", tag="z")
                for kc in range(4):
                    nc.tensor.matmul(out=f_ps[:],
                                     lhsT=fc2_sb[kc][:, m2 * 128:(m2 + 1) * 128],
                                     rhs=g1[:, kc:kc + 1],
                                     start=(kc == 0), stop=(kc == 3),
                                     skip_group_check=True)
                nc.scalar.activation(out=g2[:, m2:m2 + 1], in_=f_ps[:],
                                     func=AF.Prelu, bias=b7_sb[:, m2:m2 + 1],
                                     scale=s7_sb[:, m2:m2 + 1], alpha=0.2)

            fc3_sb = [constp.tile([128, 10], f32, name=f"fc3_sb{kc}")
                      for kc in range(2)]
            for kc in range(2):
                nc.sync.dma_start(out=fc3_sb[kc][:],
                                  in_=fc3_d[kc * 128:(kc + 1) * 128, :])
            b3_sb = constp.tile([10, 1], f32, name="b3_sb")
            nc.sync.dma_start(out=b3_sb[:], in_=b3_d[:])
            l_ps = psum.tile([10, 1], f32, name="l_ps", tag="z")
            for kc in range(2):
                nc.tensor.matmul(out=l_ps[:], lhsT=fc3_sb[kc][:],
                                 rhs=g2[:, kc:kc + 1],
                                 start=(kc == 0), stop=(kc == 1),
                                 skip_group_check=True)
            out_sb = constp.tile([10, 1], f32, name="out_sb")
            nc.scalar.activation(out=out_sb[:], in_=l_ps[:],
                                 func=AF.Identity, bias=b3_sb[:], scale=1.0)
            nc.sync.dma_start(out=out_d[:], in_=out_sb[:])

    nc.compile()
    return nc


def _prep_shared(params):
    """Host-side weight prep -> dict of numpy arrays for the bass program."""
    f = np.float32
    shared = {}

    def bn_fold(bn):
        g, b, m, v = [np.asarray(a, np.float64) for a in bn]
        s = g / np.sqrt(v + EPS)
        beta = b - m * s
        return s.astype(f), beta.astype(f)

    def col128(v):
        O = v.shape[0]
        ncol = (O + 127) // 128
        out = np.zeros((128, ncol), f)
        for p in range(ncol):
            op = min(128, O - p * 128)
            out[0:op, p] = v[p * 128:p * 128 + op]
        return out

    wkeys = [("w1", "bn1"), ("w2", "bn2"), ("w3", "bn3"), ("w4", "bn4")]
    for t, (wk, bk) in enumerate(wkeys):
        w = np.asarray(params[wk], f)          # [O, 2C]
        C = w.shape[1] // 2
        wa = w[:, :C]
        wbma = w[:, C:] - wa
        shared[f"wa{t}"] = np.ascontiguousarray(wa.T)
        shared[f"wb{t}"] = np.ascontiguousarray(wbma.T)
        s, beta = bn_fold(params[bk])
        shared[f"sc{t}"] = col128(s)
        shared[f"bi{t}"] = col128(beta)

    shared["w5t"] = np.ascontiguousarray(np.asarray(params["w5"], f).T)  # [512,1024]
    s5, b5 = bn_fold(params["bn5"])
    shared["s5"] = s5.reshape(8, 128).T.copy()
    shared["b5"] = b5.reshape(8, 128).T.copy()

    fc1 = np.asarray(params["fc1"], f).copy()  # [512, 2048]
    fc1[:, 1024:] *= 1.0 / N                   # fold mean-pool divisor
    shared["fc1t"] = np.ascontiguousarray(fc1.T)  # [2048, 512]
    s6, b6 = bn_fold(params["bn6"])
    shared["s6"] = s6.reshape(4, 128).T.copy()
    shared["b6"] = b6.reshape(4, 128).T.copy()

    shared["fc2t"] = np.ascontiguousarray(np.asarray(params["fc2"], f).T)
    s7, b7 = bn_fold(params["bn7"])
    shared["s7"] = s7.reshape(2, 128).T.copy()
    shared["b7"] = b7.reshape(2, 128).T.copy()

    shared["fc3t"] = np.ascontiguousarray(np.asarray(params["fc3_w"], f).T)
    shared["fc3b"] = np.asarray(params["fc3_b"], f).reshape(10, 1).copy()
    return shared


def kernel(x, params):
    from concourse import bass_utils

    x = np.asarray(x, np.float32)
    B = x.shape[0]
    assert x.shape == (8, N, 3)

    if "nc" not in _CACHE:
        _CACHE["nc"] = _build_program()
    nc = _CACHE["nc"]

    shared = _prep_shared(params)
    in_maps = [dict(shared, x=np.ascontiguousarray(x[i])) for i in range(B)]
    res = bass_utils.run_bass_kernel_spmd(nc, in_maps, core_ids=list(range(B)))
    return np.stack([r["out"].reshape(10) for r in res.results])
